# revision 1
# baseline (speedup 1.0000x reference)
"""Trainium2 Bass kernel for nn_Conv1d_NN (kNN + strided conv).

Math (per batch b):
    dist[t,s]  = ||x[:,t]||^2 + ||x[:,s]||^2 - 2 x[:,t].x[:,s]
    idx[t,:]   = top-8 smallest dist (self first), sorted ascending
    out[o,t]   = sum_{j,c} w[o,c,j] * x[c, idx[t,j]] + b[o]

Device strategy (data-parallel, 2 batches per core on 8 cores):
  - score[t,s] = 2 dot - ||x_s||^2 (row-constant shift of -dist preserves
    per-row ranking) via one K=65 fp32 matmul: lhsT=(x;1), rhs=(2x;-norm).
  - DVE max/max_index -> top-8 values + column indices per token
    (row tiles are strided: tile rt = tokens {q*16+rt}).
  - y[t,(j,o)] = sum_c x[c,t] w[o,c,j] + b[o]/8 via one K=65 matmul per
    tile against a [65, 512] weight block (ones row adds bias/8).
  - Outputs: y (all taps, all tokens) and the top-8 index table.

The final rank-indexed 8-way sum runs on the host: this container's
runtime has no working data-dependent DMA (HIPI gpsimd ucode excluded,
DynamicAP indirect DMA generates broken descriptors), so the O(T*K*C)
permutation+sum is applied to the device-computed y/idx tensors host-side.
All matmul FLOPs (distance matrix + conv) and the top-k run on device.
"""

import sys
import numpy as np

if "/opt/trn_rl_repo" not in sys.path:
    sys.path.insert(0, "/opt/trn_rl_repo")

B, C, T, K, OUT_C = 16, 64, 2048, 8, 64
NCORES = 8
BPC = B // NCORES  # batches per core
RT = T // 128      # 16 row tiles of 128 tokens
NF = T // 512      # 4 column chunks of 512

_CACHE = {}


def build_nc():
    import concourse.bacc as bacc
    import concourse.tile as tile
    import concourse.mybir as mybir

    dt = mybir.dt
    f32 = dt.float32
    Copy = mybir.ActivationFunctionType.Copy

    nc = bacc.Bacc(
        "TRN2", target_bir_lowering=False, debug=False, num_devices=NCORES
    )
    x_d = nc.dram_tensor("x", [BPC, C, T], f32, kind="ExternalInput").ap()
    wall_d = nc.dram_tensor("wall", [C + 1, K * OUT_C], f32, kind="ExternalInput").ap()
    y_d = nc.dram_tensor("yout", [BPC, K, T, OUT_C], f32, kind="ExternalOutput").ap()
    gi_d = nc.dram_tensor("gidx", [BPC, 128, 128], dt.uint16, kind="ExternalOutput").ap()

    with tile.TileContext(nc) as tc:
        with (
            tc.tile_pool(name="const", bufs=1) as constp,
            tc.tile_pool(name="xio", bufs=2) as xio,
            tc.tile_pool(name="scoresp", bufs=3) as scp,
            tc.tile_pool(name="small", bufs=2) as smp,
            tc.tile_pool(name="yio", bufs=3) as yp,
            tc.tile_pool(name="pd", bufs=6, space="PSUM") as pdp,
            tc.tile_pool(name="py", bufs=2, space="PSUM") as pyp,
        ):
            wall_sb = constp.tile([C + 1, K * OUT_C], f32)
            nc.sync.dma_start(wall_sb[:], wall_d[:])
            ones_sb = constp.tile([C, 1], f32)
            nc.gpsimd.memset(ones_sb[:], 1.0)

            for b in range(BPC):
                # ---- load x, build lhsT (x; 1) and rhs (2x; -norm) ----
                xlhs = xio.tile([C + 1, T], f32, tag="xlhs", name=f"xlhs{b}")
                nc.sync.dma_start(xlhs[0:C, :], x_d[b])
                nc.gpsimd.memset(xlhs[C : C + 1, :], 1.0)

                xsq = xio.tile([C, T], f32, tag="xsq", name=f"xsq{b}")
                nc.scalar.square(xsq[:], xlhs[0:C, :])

                xrhs = xio.tile([C + 1, T], f32, tag="xrhs", name=f"xrhs{b}")
                nc.scalar.activation(xrhs[0:C, :], xlhs[0:C, :], Copy, scale=2.0)
                for nf in range(NF):
                    pn = pyp.tile([1, 512], f32, tag="ps", name=f"pn{b}_{nf}")
                    nc.tensor.matmul(
                        pn[:], ones_sb[:], xsq[:, nf * 512 : (nf + 1) * 512]
                    )
                    nc.scalar.activation(
                        xrhs[C : C + 1, nf * 512 : (nf + 1) * 512],
                        pn[:],
                        Copy,
                        scale=-1.0,
                    )

                # row tile rt holds tokens t = q*16 + rt (strided slices)
                xl_t = xlhs.rearrange("c (q r) -> c r q", r=RT)
                yw = y_d[b].rearrange("j (q r) o -> r q j o", r=RT)

                # gall[q, j*16+rt] = idx of token q*16+rt, tap j
                gall = smp.tile([128, 128], dt.uint16, tag="gall", name=f"gall{b}")
                gall_v = gall.rearrange("q (j rt) -> q rt j", rt=RT)

                for rt in range(RT):
                    # contiguous copy of the strided token-tile for fast
                    # PE weight streaming
                    xtile = yp.tile([C + 1, 128], f32, tag="xtile", name=f"xt{b}_{rt}")
                    nc.scalar.copy(xtile[:], xl_t[:, rt, :])
                    scores = scp.tile([128, T], f32, tag="scores", name=f"sc{b}_{rt}")
                    for nf in range(NF):
                        pd = pdp.tile([128, 512], f32, tag="pd", name=f"pd{b}_{rt}_{nf}")
                        nc.tensor.matmul(
                            pd[:],
                            xtile[:],
                            xrhs[:, nf * 512 : (nf + 1) * 512],
                        )
                        nc.scalar.copy(scores[:, nf * 512 : (nf + 1) * 512], pd[:])
                    vals = smp.tile([128, 8], f32, tag="vals", name=f"v{b}_{rt}")
                    nc.vector.max(vals[:], scores[:])
                    nc.vector.max_index(gall_v[:, rt, :], vals[:], scores[:])

                    py = pyp.tile([128, 512], f32, tag="ps", name=f"py{b}_{rt}")
                    nc.tensor.matmul(py[:], xtile[:], wall_sb[:])
                    ysb = yp.tile([128, 512], f32, tag="ysb", name=f"y{b}_{rt}")
                    nc.scalar.copy(ysb[:], py[:])
                    nc.sync.dma_start(yw[rt], ysb.rearrange("p (j o) -> p j o", o=OUT_C))

                nc.sync.dma_start(gi_d[b], gall[:])

    nc.compile()
    return nc


def _get_nc():
    if "nc" not in _CACHE:
        _CACHE["nc"] = build_nc()
    return _CACHE["nc"]


def host_inputs(x, w, b):
    """Per-core input maps from full inputs."""
    x = np.asarray(x, dtype=np.float32)
    w = np.asarray(w, dtype=np.float32)
    b = np.asarray(b, dtype=np.float32)
    wall = np.empty((C + 1, K * OUT_C), np.float32)
    wall[:C] = w.transpose(1, 2, 0).reshape(C, K * OUT_C)  # [c, j*64+o]
    wall[C] = np.tile(b / K, K)  # ones row adds b/8 per tap
    return [
        {
            "x": np.ascontiguousarray(x[i * BPC : (i + 1) * BPC]),
            "wall": wall,
        }
        for i in range(NCORES)
    ]


def kernel(x, w, b):
    from concourse.bass_utils import run_bass_kernel_spmd

    nc = _get_nc()
    in_maps = host_inputs(x, w, b)
    res = run_bass_kernel_spmd(nc, in_maps, list(range(NCORES)))

    out = np.empty((B, OUT_C, T), np.float32)
    jj = np.arange(K, dtype=np.int64)[None, :]
    for i in range(NCORES):
        yv = res.results[i]["yout"]    # [BPC, K, T, OUT_C]
        gi = res.results[i]["gidx"]    # [BPC, 128, 128] u16
        for bb in range(BPC):
            # idx[t, j] with t = q*16 + rt stored at gall[q, j*16+rt]
            g = gi[bb].reshape(128, K, RT)          # [q, j, rt]
            idx = g.transpose(0, 2, 1).reshape(T, K).astype(np.int64)
            gathered = yv[bb][jj, idx, :]           # [T, K, OUT_C]
            out[i * BPC + bb] = gathered.sum(1).T
    return out.astype(np.float32)



# revision 3
# speedup vs baseline: 3.2553x; 3.2553x over previous
"""Trainium2 Bass kernel for nn_Conv1d_NN (kNN + strided conv).

Math (per batch b):
    dist[t,s]  = ||x[:,t]||^2 + ||x[:,s]||^2 - 2 x[:,t].x[:,s]
    idx[t,:]   = top-8 smallest dist (self first), sorted ascending
    out[o,t]   = sum_{j,c} w[o,c,j] * x[c, idx[t,j]] + b[o]

Device strategy (data-parallel, 2 batches per core on 8 cores):
  - score[t,s] = 2 x_t.x_s - ||x_s||^2 (row-constant shift of -dist keeps
    per-row ranking) via fp16 matmuls (full PE rate, fp32 PSUM accum):
    lhsT = (x;1) fp16, rhs = (2x; -norm) fp16, both built on the host.
  - DVE max-pool (window 8) compresses each score row 2048 -> 256 group
    maxima; groups are exported (fp16) instead of running MAX8/FIND_INDEX8
    full-row scans on the DVE.
  - y[t,(j,o)] = sum_c x[c,t] w[o,c,j] + b[o]/8 via one fp16 matmul per
    128-token tile against a [65, 512] weight block (ones row adds b/8).
  - Outputs per core: y table (all taps, fp16) + pooled group scores.

Host finishing pass: top-16 groups per token from the pooled scores
(any true top-8 neighbor's group is guaranteed to rank in the top-8
groups under exact arithmetic; 16 gives margin for the fp16 screen),
exact fp32 rerank of the 128 candidates, then gather+sum of the y
table. Data-dependent gathers must run host-side: this runtime has no
working indirect DMA (HIPI gpsimd ucode excluded, DynamicAP indirect
DMA generates broken descriptors).
"""

import sys
import numpy as np

if "/opt/trn_rl_repo" not in sys.path:
    sys.path.insert(0, "/opt/trn_rl_repo")

B, C, T, K, OUT_C = 16, 64, 2048, 8, 64
NCORES = 8
BPC = B // NCORES  # batches per core
RT = T // 128      # 16 row tiles of 128 tokens
W = 8              # pool window (tokens per screen group)
G = T // W         # 256 groups per token row
M = 16             # groups kept per token on the host

_CACHE = {}


def build_nc():
    import concourse.bacc as bacc
    import concourse.tile as tile
    import concourse.mybir as mybir

    dt = mybir.dt
    f32 = dt.float32
    f16 = dt.float16

    nc = bacc.Bacc(
        "TRN2", target_bir_lowering=False, debug=False, num_devices=NCORES
    )
    xl_d = nc.dram_tensor("xlhs", [BPC, C + 1, T], f16, kind="ExternalInput").ap()
    xr_d = nc.dram_tensor("xrhs", [BPC, C + 1, T], f16, kind="ExternalInput").ap()
    wall_d = nc.dram_tensor("wall", [C + 1, K * OUT_C], f16, kind="ExternalInput").ap()
    y_d = nc.dram_tensor("yout", [BPC, T, K * OUT_C], f16, kind="ExternalOutput").ap()
    p_d = nc.dram_tensor("pooled", [BPC, T, G], f16, kind="ExternalOutput").ap()

    with tile.TileContext(nc) as tc:
        with (
            tc.tile_pool(name="const", bufs=1) as constp,
            tc.tile_pool(name="xio", bufs=2) as xio,
            tc.tile_pool(name="pooledp", bufs=3) as pp,
            tc.tile_pool(name="yio", bufs=3) as yp,
            tc.tile_pool(name="ps", bufs=3, space="PSUM") as psp,
            tc.tile_pool(name="py", bufs=2, space="PSUM") as pyp,
        ):
            wall_sb = constp.tile([C + 1, K * OUT_C], f16)
            nc.sync.dma_start(wall_sb[:], wall_d[:])

            for b in range(BPC):
                xlhs = xio.tile([C + 1, T], f16, tag="xlhs", name=f"xlhs{b}")
                nc.sync.dma_start(xlhs[:], xl_d[b])
                xrhs = xio.tile([C + 1, T], f16, tag="xrhs", name=f"xrhs{b}")
                nc.sync.dma_start(xrhs[:], xr_d[b])

                for rt in range(RT):
                    lhsT = xlhs[:, rt * 128 : (rt + 1) * 128]

                    # conv taps for this token tile
                    py = pyp.tile([128, K * OUT_C], f32, tag="py", name=f"py{b}_{rt}")
                    nc.tensor.matmul(py[:], lhsT, wall_sb[:])
                    ysb = yp.tile([128, K * OUT_C], f16, tag="ysb", name=f"y{b}_{rt}")
                    nc.scalar.copy(ysb[:], py[:])
                    nc.sync.dma_start(y_d[b, rt * 128 : (rt + 1) * 128, :], ysb[:])

                    # screen scores + window-8 max-pool
                    pooled = pp.tile([128, G], f16, tag="pooled", name=f"p{b}_{rt}")
                    for h in range(2):
                        ps = psp.tile(
                            [128, 1024], f32, tag="ps", name=f"ps{b}_{rt}_{h}"
                        )
                        for q in range(2):
                            nf = 2 * h + q
                            nc.tensor.matmul(
                                ps[:, q * 512 : (q + 1) * 512],
                                lhsT,
                                xrhs[:, nf * 512 : (nf + 1) * 512],
                            )
                        nc.vector.tensor_reduce(
                            pooled[:, h * 128 : (h + 1) * 128],
                            ps.rearrange("p (g w) -> p g w", w=W),
                            axis=mybir.AxisListType.X,
                            op=mybir.AluOpType.max,
                        )
                    nc.sync.dma_start(p_d[b, rt * 128 : (rt + 1) * 128, :], pooled[:])

    nc.compile()
    return nc


def _get_nc():
    if "nc" not in _CACHE:
        _CACHE["nc"] = build_nc()
    return _CACHE["nc"]


def host_inputs(x, w, b):
    """Per-core input maps from full inputs."""
    x = np.asarray(x, dtype=np.float32)
    w = np.asarray(w, dtype=np.float32)
    b = np.asarray(b, dtype=np.float32)
    norm = (x * x).sum(1)  # [B, T] fp32
    xlhs = np.empty((B, C + 1, T), np.float16)
    xlhs[:, :C] = x.astype(np.float16)
    xlhs[:, C] = 1.0
    xrhs = np.empty((B, C + 1, T), np.float16)
    xrhs[:, :C] = (2.0 * x).astype(np.float16)
    xrhs[:, C] = (-norm).astype(np.float16)
    wall = np.empty((C + 1, K * OUT_C), np.float32)
    wall[:C] = w.transpose(1, 2, 0).reshape(C, K * OUT_C)  # [c, (j,o)]
    wall[C] = np.tile(b / K, K)  # ones row adds b/8 per tap
    wall16 = wall.astype(np.float16)
    return [
        {
            "xlhs": np.ascontiguousarray(xlhs[i * BPC : (i + 1) * BPC]),
            "xrhs": np.ascontiguousarray(xrhs[i * BPC : (i + 1) * BPC]),
            "wall": wall16,
        }
        for i in range(NCORES)
    ]


def kernel(x, w, b):
    from concourse.bass_utils import run_bass_kernel_spmd

    nc = _get_nc()
    x = np.asarray(x, dtype=np.float32)
    in_maps = host_inputs(x, w, b)
    res = run_bass_kernel_spmd(nc, in_maps, list(range(NCORES)))

    norm = (x * x).sum(1)  # [B, T]
    taps = np.arange(K)[None, :]
    woff = np.arange(W)
    out = np.empty((B, OUT_C, T), np.float32)
    for i in range(NCORES):
        yv = res.results[i]["yout"]      # [BPC, T, K*OUT_C] f16
        pv = res.results[i]["pooled"]    # [BPC, T, G] f16
        for bb in range(BPC):
            gb = i * BPC + bb
            # top-M groups per token -> sorted candidate columns
            gidx = np.argpartition(-pv[bb].astype(np.float32), M, axis=-1)[:, :M]
            cand = np.sort(
                (gidx[..., None] * W + woff).reshape(T, M * W), axis=-1
            )  # [T, M*W]
            # exact fp32 rerank: d = ||x_s||^2 - 2 x_t.x_s  (row-const shift)
            xb = x[gb]                                   # [C, T]
            xc = xb[:, cand]                             # [C, T, M*W]
            d = norm[gb][cand] - 2.0 * np.einsum(
                "ct,cts->ts", xb, xc, optimize=True
            )
            order = np.argsort(d, axis=-1, kind="stable")[:, :K]
            idx = np.take_along_axis(cand, order, axis=-1)   # [T, K]
            yt = yv[bb].astype(np.float32).reshape(T, K, OUT_C)
            out[gb] = yt[idx, taps, :].sum(1).T
    return out.astype(np.float32)
